# revision 54
# baseline (speedup 1.0000x reference)
"""Trainium2 Bass kernel: causal multi-head self-attention with RoPE.

Problem: B=4, S=2048, D=1024, H=16, DK=64.  out = softmax(causal(qk^T/8)) v @ wo^T
with q,k RoPE-rotated.

Sharding: 8 cores = (batch b in 0..3) x (head-group g in 0..1, 8 heads each).
Each core computes its batch's QKV for its 8 heads, causal attention, and a
partial output projection; the host sums the two group-partials per batch.

Schedule: qc-major block order (all pairs at qc=0, then qc=1, ...) so the
output projection for each qc unlocks at the 25/50/75% marks and filler work
is spread across the whole kernel instead of drying up after pair 1.  The
per-chunk chain scores->exp->mask->AV is pipelined with the AV emitted three
chunks behind its scores and carried ACROSS block boundaries (a global pend
queue), so the scalar-engine exp hides behind PE work with no clock model.
Filler (v/q/k projections, rope, output projection) is paced by a quota:
after every chunk the emitter drains filler units until the global
done-count matches the chunk fraction; the rope of the next two blocks is
pre-pulled mid-block so block starts never wait on the rope DMA+DVE chain.

Key perf mechanisms found by tracing:
- softmax denominators: the ones-column rides the AV stationary into PSUM
  row 64; the 1/den broadcast is built WITHOUT any DMA roundtrip (the old
  [1,512]->[64,512] broadcast DMA took 6-11us serializing on one DMA
  engine): the ov bank is evacuated by a single ScalarE copy (ACT reads
  PSUM fast and is ~50% idle), then DVE stream_shuffle broadcasts the
  denominator row to a base-0 window, reciprocal_approx_fast, and a
  quadrant-copy shuffle build [64,512] of 1/den.  DVE compute ops cannot
  cross base partitions (verified: garbage results), shuffles can.
- diagonal trim: the two upper diagonal k-chunks of every block have
  q<256 fully masked, so scores/exp/mask/AV compute only the [256:512)
  q-window (the two heads' trimmed windows stay in separate PSUM banks --
  two concurrent row-tiled matmuls writing one bank locks the device).
- input DMAs are ~20 large transfers with 2-8KB contiguous partition lines
  (DMA rate is line-count-bound; dma_start issue is ~0.7us of queue time),
  in first-use order across sync/scalar/gpsimd queues; q/k weights are
  staged pair-major so pair 0's 0.25MB slice lands first and the first
  projection starts ~10us after the framework preamble.
- the tail output projection is split: jc 0-2 accumulate to SBUF while the
  last block runs; only 8 jc=3 matmuls wait on the final evac chain, and a
  reserved stash of outproj units keeps the PE warm (HAM at 2.4GHz) and
  busy under that chain.
"""
import os
import sys

for _p in ("/opt/trn_rl_repo", "/root/.axon_site/_ro/trn_rl_repo"):
    if os.path.isdir(_p) and _p not in sys.path:
        sys.path.insert(0, _p)

import numpy as np
import ml_dtypes

import concourse.bass as bass
import concourse.mybir as mybir
import concourse.tile as tile
from concourse import bacc
from concourse.bass_utils import run_bass_kernel_spmd

B, S, D, H = 4, 2048, 1024, 16
DK = D // H          # 64
HG = 8               # heads per group
NG = 2               # head groups (cores per batch)
THETA = 10000.0
NCORES = 8

BF16 = mybir.dt.bfloat16
F32 = mybir.dt.float32
bf16 = ml_dtypes.bfloat16

QT = 512             # q tile width (free dim)
NQT = S // QT        # 4
NKT = S // 128       # 16 k chunks
NJT = HG * DK // 128  # 4 j-tiles (head pairs)
NDC = D // 128       # 8 d chunks
NMT = D // 128       # 8 output m tiles
NCHUNK = sum(4 * qc + 4 for qc in range(NQT)) * NJT   # 160


def _build_nc():
    from collections import deque

    nc = bacc.Bacc("TRN2", target_bir_lowering=False, debug=False)
    # DRAM layouts are SBUF-native [128, chunk, free] so input DMAs are a few
    # large 3D-AP transfers instead of ~100 small ones.
    # token-major x so every DMA slice is contiguous per partition row
    xT = nc.dram_tensor("xT", [NQT, 128, NDC, QT], BF16,
                        kind="ExternalInput").ap()
    # pair-major q/k weights: pair 0's 0.25MB slice lands early so the
    # first projections + rope start ~10us sooner
    wqT = nc.dram_tensor("wqT", [NJT, 128, NDC, 128], BF16,
                         kind="ExternalInput").ap()
    wkT = nc.dram_tensor("wkT", [NJT, 128, NDC, 128], BF16,
                         kind="ExternalInput").ap()
    wvT = nc.dram_tensor("wvT", [128, NDC, HG * DK], BF16,
                         kind="ExternalInput").ap()
    woT = nc.dram_tensor("woT", [128, NJT, D], BF16, kind="ExternalInput").ap()
    c128 = nc.dram_tensor("c128", [128, S], BF16, kind="ExternalInput").ap()
    s128 = nc.dram_tensor("s128", [128, S], BF16, kind="ExternalInput").ap()
    maskd = nc.dram_tensor("maskd", [128, 4, QT], BF16, kind="ExternalInput").ap()
    outT = nc.dram_tensor("outT", [128, NMT, S], BF16, kind="ExternalOutput").ap()

    from contextlib import ExitStack
    with tile.TileContext(nc) as tc, ExitStack() as stk:
        pp = stk.enter_context(tc.tile_pool(name="persist", bufs=1))
        ep = stk.enter_context(tc.tile_pool(name="epool", bufs=6))
        sp = stk.enter_context(tc.tile_pool(name="smalls", bufs=2))
        qw = stk.enter_context(tc.tile_pool(name="qkvwork", bufs=2))
        ps_st = stk.enter_context(
            tc.tile_pool(name="ps_st", bufs=2, space="PSUM"))
        ps_ov = stk.enter_context(
            tc.tile_pool(name="ps_ov", bufs=2, space="PSUM"))
        ps_qkv = stk.enter_context(
            tc.tile_pool(name="ps_qkv", bufs=2, space="PSUM"))

        # ---------------- persistent tiles ----------------
        wo_sb = pp.tile([128, NJT, D], BF16)
        m_sb = pp.tile([128, 4, QT], BF16)
        qrot = pp.tile([128, NJT, S], BF16)
        krot = pp.tile([128, NJT, S], BF16)
        v_aug = pp.tile([128, NKT, HG, 66], BF16)
        a_t = pp.tile([128, NJT, S], BF16)
        # token-quarter-major x: [:, t, dc, :] slices are contiguous per
        # partition on both the DRAM and SBUF side (8KB lines -> fast DMA)
        xT_sb = pp.tile([128, NQT, NDC, QT], BF16)
        wq_sb = pp.tile([128, NJT, NDC, 128], BF16)
        wk_sb = pp.tile([128, NJT, NDC, 128], BF16)
        wv_sb = pp.tile([128, NDC, HG * DK], BF16)
        c_sb = pp.tile([128, S], BF16)
        s_sb = pp.tile([128, S], BF16)

        nc.gpsimd.memset(v_aug[:, :, :, 64:65], 1.0)

        # ------- input DMAs: few big transfers, ordered by first use -------
        # sync carries dc 0-3, scalar dc 4-7 (scalar queue is free until the
        # first exp); gpsimd carries only small latency-critical transfers.
        nc.gpsimd.dma_start(c_sb[:], c128[:])
        nc.gpsimd.dma_start(s_sb[:], s128[:])
        nc.gpsimd.dma_start(m_sb[:], maskd[:])
        # few large transfers; per-partition lines are 4-8KB contiguous.
        # wq/x-t0 interleave across both queues so the first projection
        # matmuls (dc-pair granularity) unblock as early as possible.
        nc.scalar.dma_start(wq_sb[:, 0], wqT[0])
        nc.scalar.dma_start(wk_sb[:, 0], wkT[0])
        nc.sync.dma_start(xT_sb[:, 0, 0:4, :], xT[0, :, 0:4, :])
        nc.sync.dma_start(xT_sb[:, 0, 4:8, :], xT[0, :, 4:8, :])
        nc.scalar.dma_start(wq_sb[:, 1], wqT[1])
        nc.scalar.dma_start(wk_sb[:, 1], wkT[1])
        nc.sync.dma_start(xT_sb[:, 1, 0:4, :], xT[1, :, 0:4, :])
        nc.sync.dma_start(xT_sb[:, 1, 4:8, :], xT[1, :, 4:8, :])
        nc.scalar.dma_start(wq_sb[:, 2], wqT[2])
        nc.scalar.dma_start(wk_sb[:, 2], wkT[2])
        nc.scalar.dma_start(wq_sb[:, 3], wqT[3])
        nc.scalar.dma_start(wk_sb[:, 3], wkT[3])
        nc.sync.dma_start(wv_sb[:], wvT[:])
        nc.scalar.dma_start(xT_sb[:, 2], xT[2])
        nc.sync.dma_start(xT_sb[:, 3], xT[3])
        nc.scalar.dma_start(wo_sb[:], woT[:])

        # ---------------- emit helpers ----------------
        def proj_mms(pair, w_sb, ps, tn, dlo, dhi):
            for dc in range(dlo, dhi):
                nc.tensor.matmul(
                    ps[:],
                    w_sb[:, pair, dc, :],
                    xT_sb[:, tn, dc, :],
                    start=(dc == 0), stop=(dc == NDC - 1))

        def rope_dma(pair, name, pre, hf, dma_eng=None):
            # stage the 32-block partition swap of pre (DMA only, no DVE);
            # issued ~8 filler units before rope_mul so the DVE never
            # head-of-line blocks on this transfer.
            swp = qw.tile([128, 1024], BF16, tag="swp" + name,
                          name=f"swp{name}{pair}{hf}")
            for a in range(4):
                lo, sw = 32 * a, 32 * (a ^ 1)
                (dma_eng or nc.gpsimd).dma_start(
                    swp[lo:lo + 32, :], pre[sw:sw + 32, :])
            return swp

        def rope_mul(pair, pre, swp, dst, hf):
            cs = slice(hf * 1024, (hf + 1) * 1024)
            nc.vector.tensor_mul(dst[:, pair, cs], pre[:], c_sb[:, cs])
            nc.vector.tensor_mul(swp[:], swp[:], s_sb[:, cs])
            nc.vector.tensor_add(dst[:, pair, cs], dst[:, pair, cs], swp[:])

        def emit_scores(pair, qc, kc):
            # one k-chunk, both heads of the pair packed into one st tile.
            # Full chunks: st[:, 0:512] = head 0, st[:, 512:1024] = head 1.
            # Upper-diagonal chunks (par>=2) have q < 256 fully masked, so
            # only the q-window [256:512) is computed, packed as
            # st[:, 0:256] = head 0, st[:, 256:512] = head 1.
            par = kc - 4 * qc
            st = ps_st.tile([128, 2 * QT], F32, tag="st",
                            name=f"st_{pair}{qc}{kc}")
            w = 256 if par >= 2 else QT
            qlo = qc * QT + (QT - w)
            for h01 in range(2):
                lo = 64 * h01
                nc.tensor.matmul(
                    st[:, h01 * QT:h01 * QT + w],
                    krot[lo:lo + 64, pair, kc * 128:(kc + 1) * 128],
                    qrot[lo:lo + 64, pair, qlo:(qc + 1) * QT],
                    start=True, stop=True,
                    tile_position=(lo, 0))
            return st

        def emit_exp(pair, qc, kc, st):
            par = kc - 4 * qc
            e = ep.tile([128, 2 * QT], BF16, tag="e", bufs=5,
                        name=f"e{pair}{qc}{kc}")
            w = 256 if par >= 2 else QT
            if w == QT:
                nc.scalar.activation(
                    e[:], st[:], mybir.ActivationFunctionType.Exp,
                    scale=0.125)
            else:
                # one strided ACT op covering both heads' trimmed windows
                ev = e[:].rearrange("p (h x) -> p h x", h=2)[:, :, 0:w]
                sv = st[:].rearrange("p (h x) -> p h x", h=2)[:, :, 0:w]
                nc.scalar.activation(
                    ev, sv, mybir.ActivationFunctionType.Exp, scale=0.125)
            if par >= 0:        # diagonal band: mask both heads' windows
                for h01 in range(2):
                    nc.vector.tensor_mul(
                        e[:, h01 * QT:h01 * QT + w],
                        e[:, h01 * QT:h01 * QT + w],
                        m_sb[:, par, QT - w:QT])
            return e

        def emit_av(pair, qc, kc, e, ov0, ov1, last):
            par = kc - 4 * qc
            w = 256 if par >= 2 else QT
            for h01, ov in ((0, ov0), (1, ov1)):
                nc.tensor.matmul(
                    ov[0:65, QT - w:QT],
                    v_aug[:, kc, 2 * pair + h01, 0:65],
                    e[:, h01 * QT:h01 * QT + w],
                    start=(kc == 0),
                    stop=last, skip_group_check=True)

        def emit_evac_a(pair, qc, ov0, ov1):
            """Free the ov banks and build the 1/den broadcast entirely on
            the DVE: broadcast-shuffle the PSUM denominator row down to a
            base-0 window, reciprocal there, quadrant-copy to 64 rows.  No
            DMA roundtrips, so the deferred normalize (emit_evac_b) never
            head-of-line blocks the DVE FIFO."""
            ous = []
            for h01, ov in ((0, ov0), (1, ov1)):
                # single scalar-engine copy (incl. the denominator row) is
                # the only reader of the ov bank, so the bank frees ~1.5us
                # after the last AV regardless of DVE queue depth.  The DVE
                # reciprocal-broadcast chain is deferred to emit_evac_b so
                # it never sits ahead of rope/mask work at block boundaries.
                ou = ep.tile([96, QT], F32, tag="ou", bufs=3,
                             name=f"ou{pair}{qc}{h01}")
                nc.scalar.copy(ou[0:65, :], ov[0:65, :])
                ous.append(ou)
            return ous, None

        def emit_evac_b(pair, qc, ous, rbs):
            rbs = []
            for h01 in range(2):
                den = sp.tile([32, QT], F32, tag="den", bufs=2,
                              name=f"den{pair}{qc}{h01}")
                nc.vector.stream_shuffle(den[:], ous[h01][64:96, :],
                                         mask=[0] * 32)
                rb = sp.tile([64, QT], F32, tag="rb", bufs=4,
                             name=f"rb{pair}{qc}{h01}")
                nc.vector.reciprocal_approx_fast(rb[0:32, :], den[:])
                nc.vector.stream_shuffle(rb[32:64, :], rb[0:32, :],
                                         mask=list(range(32)))
                rbs.append(rb)
            nc.vector.tensor_mul(
                a_t[0:64, pair, qc * QT:(qc + 1) * QT],
                ous[0][0:64, :], rbs[0][:])
            an = sp.tile([64, QT], BF16, tag="an", bufs=3,
                         name=f"an{pair}{qc}")
            nc.vector.tensor_mul(an[:], ous[1][0:64, :], rbs[1][:])
            nc.gpsimd.dma_start(
                a_t[64:128, pair, qc * QT:(qc + 1) * QT], an[:])

        # ---------------- filler unit machinery (no clock model) ----------
        filler = deque()     # items: (label, fn)
        emitted = set()
        state = {"done": 0}
        TOTAL_UNITS = (NKT * 4) + (NJT * 2 * 20) + (3 * NMT * 2) + 24  # 296

        def run_one():
            lab, fn = filler.popleft()
            fn()
            state["done"] += 1
            if lab is not None:
                emitted.add(lab)

        def drain_until(lab):
            while lab not in emitted:
                assert filler, f"filler ran dry before {lab}"
                run_one()

        def pace(frac):
            tgt = frac * TOTAL_UNITS
            while filler and state["done"] < tgt:
                run_one()

        # --- v projection units: 16 token-tiles, 4 chunks of 2 dc each ---
        def v_units(tlo, thi):
            for tt in range(tlo, thi):
                cell = {}

                def mk(tt, dlo, dhi, last):
                    def fn():
                        if dlo == 0:
                            cell["ps"] = ps_qkv.tile(
                                [128, QT], F32, tag="qv", name=f"psv{tt}")
                        ps = cell["ps"]
                        for dc in range(dlo, dhi):
                            nc.tensor.matmul(
                                ps[:],
                                xT_sb[:, tt // 4, dc,
                                      (tt % 4) * 128:(tt % 4 + 1) * 128],
                                wv_sb[:, dc, :],
                                start=(dc == 0), stop=(dc == NDC - 1))
                        if last:
                            nc.vector.tensor_copy(
                                v_aug[:, tt, :, 0:64],
                                ps[:].rearrange("p (h d) -> p h d", h=HG))
                    return fn
                for ci in range(4):
                    dlo, dhi = 2 * ci, 2 * ci + 2
                    lab = ("v", tt) if ci == 3 else None
                    yield (lab, mk(tt, dlo, dhi, ci == 3))

        # --- q/k projection + rope units for one (pair, half) ---
        def proj_half_units(pair, hf, swp_eng=None):
            preq = qw.tile([128, 1024], BF16, tag="preq",
                           name=f"preq{pair}{hf}")
            prek = qw.tile([128, 1024], BF16, tag="prek",
                           name=f"prek{pair}{hf}")
            swps = {}

            def mkp(w_sb, pre, tn, dlo, dhi, last, cell):
                def fn():
                    if dlo == 0:
                        cell["ps"] = ps_qkv.tile(
                            [128, QT], F32, tag="qv",
                            name=f"psp{pair}{tn}")
                    ps = cell["ps"]
                    proj_mms(pair, w_sb, ps, tn, dlo, dhi)
                    if last:
                        nc.vector.tensor_copy(
                            pre[:, (tn - 2 * hf) * QT:(tn - 2 * hf + 1) * QT],
                            ps[:])
                return fn

            def mkd(name, pre):
                def fn():
                    swps[name] = rope_dma(pair, name, pre, hf,
                                          dma_eng=swp_eng)
                return fn

            def mkm(name, pre, dst):
                def fn():
                    rope_mul(pair, pre, swps[name], dst, hf)
                return fn

            # tn-interleaved (q-tn0, k-tn0, q-tn1, k-tn1) to match the DMA
            # arrival order in the head phase (x-t0 + wq, then wk, then x-t1)
            for tn_i in range(2):
                for name, w_sb, pre in (("q", wq_sb, preq),
                                        ("k", wk_sb, prek)):
                    tn = 2 * hf + tn_i
                    cell = {}
                    for ci in range(4):
                        yield (None, mkp(w_sb, pre, tn, 2 * ci,
                                         2 * ci + 2, ci == 3, cell))
            for name, pre in (("q", preq), ("k", prek)):
                yield (None, mkd(name, pre))
            for name, pre, dst in (("q", preq, qrot), ("k", prek, krot)):
                yield (("rope" + name, pair, hf), mkm(name, pre, dst))

        # --- output projection units for one qc: 8 mt, 2 chunks of 2 jc ---
        def outproj_units(qc):
            for mt in range(NMT):
                cell = {}

                def mk(qc, mt, jlo, jhi, last):
                    def fn():
                        if jlo == 0:
                            cell["ps"] = ps_qkv.tile(
                                [128, QT], F32, tag="qv", name=f"op{qc}{mt}")
                        op = cell["ps"]
                        for jc in range(jlo, jhi):
                            nc.tensor.matmul(
                                op[:],
                                wo_sb[:, jc, mt * 128:(mt + 1) * 128],
                                a_t[:, jc, qc * QT:(qc + 1) * QT],
                                start=(jc == 0), stop=(jc == NJT - 1))
                        if last:
                            ot = sp.tile([128, QT], BF16, tag="ot", bufs=3,
                                         name=f"ot{qc}{mt}")
                            nc.vector.tensor_copy(ot[:], op[:])
                            nc.sync.dma_start(
                                outT[:, mt, qc * QT:(qc + 1) * QT],
                                ot[:])
                    return fn
                for ci in range(2):
                    yield (None, mk(qc, mt, 2 * ci, 2 * ci + 2, ci == 1))

        # final round's outproj is split: jc 0-2 accumulate to SBUF while the
        # last block still runs; only the 8 jc=3 matmuls wait on the final
        # evac chain, shrinking the kernel tail from ~8us to ~2.5us.
        ota_hold = {}

        def outproj_a_units(qc):
            for mt in range(NMT):
                def mk(qc, mt):
                    def fn():
                        op = ps_qkv.tile([128, QT], F32, tag="qv",
                                         name=f"opA{qc}{mt}")
                        for jc in range(3):
                            nc.tensor.matmul(
                                op[:],
                                wo_sb[:, jc, mt * 128:(mt + 1) * 128],
                                a_t[:, jc, qc * QT:(qc + 1) * QT],
                                start=(jc == 0), stop=(jc == 2))
                        ota = sp.tile([128, QT], F32, tag="ota", bufs=8,
                                      name=f"ota{mt}")
                        nc.vector.tensor_copy(ota[:], op[:])
                        ota_hold[mt] = ota
                    return fn
                yield (None, mk(qc, mt))

        def outproj_b_units(qc):
            for mt in range(NMT):
                def mk(qc, mt):
                    def fn():
                        op = ps_qkv.tile([128, QT], F32, tag="qv",
                                         name=f"opB{qc}{mt}")
                        nc.tensor.matmul(
                            op[:],
                            wo_sb[:, 3, mt * 128:(mt + 1) * 128],
                            a_t[:, 3, qc * QT:(qc + 1) * QT],
                            start=True, stop=True)
                        ot = sp.tile([128, QT], BF16, tag="ot", bufs=3,
                                     name=f"otB{qc}{mt}")
                        nc.vector.tensor_add(ot[:], op[:], ota_hold[mt][:])
                        nc.sync.dma_start(
                            outT[:, mt, qc * QT:(qc + 1) * QT], ot[:])
                    return fn
                yield (None, mk(qc, mt))

        # ------ phase 0: pair-0 first-half q/k projections + rope up front
        for lab, fn in proj_half_units(0, 0, swp_eng=nc.gpsimd):
            fn()
            state["done"] += 1
            if lab is not None:
                emitted.add(lab)

        # filler deque ordered so qc-major forced drains pull just-in-time:
        # round qc0 needs v tt0-3 + all pairs' half-0; round qc1 needs v
        # tt4-7; round qc2 needs half-1 rope + v tt8-11; round qc3 the rest.
        for u in v_units(0, 4):
            filler.append(u)
        for pair in (1, 2, 3):
            for u in proj_half_units(pair, 0):
                filler.append(u)
        for u in v_units(4, 8):
            filler.append(u)
        for u in proj_half_units(0, 1):
            filler.append(u)
        for u in v_units(8, 12):
            filler.append(u)
        for u in proj_half_units(1, 1):
            filler.append(u)
        for u in proj_half_units(2, 1):
            filler.append(u)
        for u in proj_half_units(3, 1):
            filler.append(u)
        for u in v_units(12, 16):
            filler.append(u)
        # outproj units are appended as each qc round's last evac_b fires

        # -------- main attention loop (qc-major, cross-block pipeline) ----
        # The AV+evac tail of each block is interleaved with the next
        # block's scores via a global pend queue, so the PE never drains
        # while the last exps of a block finish.
        hold = {"b": None, "age": 0}  # (pair, qc, ous, rbs) + chunks waited

        def fire_evac_b():
            bpair, bqc, ous, rbs = hold["b"]
            emit_evac_b(bpair, bqc, ous, rbs)
            hold["b"] = None
            if bqc == NQT - 1 and bpair == NJT - 2:
                for u in outproj_a_units(bqc):
                    filler.append(u)
            if bpair == NJT - 1:
                if bqc == NQT - 1:
                    gen = list(outproj_b_units(bqc))
                elif bqc == NQT - 2:
                    # hold back the last 6 mt so the PE stays warm (and
                    # busy) during the final block's evac chain
                    units = list(outproj_units(bqc))
                    tail_stash.extend(units[-16:])
                    gen = units[:-16]
                else:
                    gen = list(outproj_units(bqc))
                for u in gen:
                    filler.append(u)

        blocks = [(qc, pair) for qc in range(NQT) for pair in range(NJT)]
        pend = deque()   # (pair, qc, kc, e, ov0, ov1, last)
        tail_stash = []

        def flush_one():
            fpair, fqc, kc, e, ov0, ov1, last = pend.popleft()
            if fpair == 0:
                drain_until(("v", kc))
            emit_av(fpair, fqc, kc, e, ov0, ov1, last)
            if last:
                if hold["b"] is not None:   # short blocks: fire before reuse
                    fire_evac_b()
                hold["b"] = (fpair, fqc) + emit_evac_a(fpair, fqc, ov0, ov1)
                hold["age"] = 0

        gchunk = 0
        for bi, (qc, pair) in enumerate(blocks):
            ngrp = 4 * qc + 4      # k chunks covering causal range
            ov0 = ps_ov.tile([96, QT], F32, tag="ov", name=f"ov0_{pair}{qc}")
            ov1 = ps_ov.tile([96, QT], F32, tag="ov", name=f"ov1_{pair}{qc}")
            drain_until(("ropeq", pair, qc // 2))
            drain_until(("ropek", pair, 0))
            for kc in range(ngrp):
                if kc >= 8:
                    drain_until(("ropek", pair, 1))
                if pair == 0:
                    drain_until(("v", min(kc + 2, ngrp - 1)))
                st = emit_scores(pair, qc, kc)
                e = emit_exp(pair, qc, kc, st)
                pend.append((pair, qc, kc, e, ov0, ov1, kc == ngrp - 1))
                # fire the deferred normalize only once its broadcast DMA
                # chain (~4 chunks) has surely landed, so the DVE FIFO never
                # head-of-line blocks the masks queued behind it
                hold["age"] += 1
                if hold["b"] is not None and hold["age"] >= 6:
                    fire_evac_b()
                gchunk += 1
                pace((gchunk - 0.5) / NCHUNK)
                if len(pend) > 3:
                    flush_one()
                pace(gchunk / NCHUNK)
                # the final block is exp-bound once regular filler runs dry;
                # feed half the reserved stash through it, keep the rest for
                # the evac-chain tail
                if bi == len(blocks) - 1 and kc % 2 == 0 and                         len(tail_stash) > 8 and not filler:
                    filler.append(tail_stash.pop(0))
                    run_one()
                if kc in (1, 2, 3) and bi + kc < len(blocks):
                    # pre-pull the rope (and projections) of the next TWO
                    # blocks so a block start never waits on the rope
                    # DMA+DVE chain, even across the short qc=0 blocks
                    nqc, npair = blocks[bi + kc]
                    drain_until(("ropeq", npair, nqc // 2))
                    drain_until(("ropek", npair, 0))
                    if nqc >= 2:
                        drain_until(("ropek", npair, 1))

        # drain the pipeline: remaining AVs with reserved filler interleaved;
        # the bulk of the reserve runs after the final evac_a so the PE stays
        # warm (and useful) under the final DVE evac chain
        while pend:
            if tail_stash:
                filler.append(tail_stash.pop(0))
            if filler:
                run_one()
            flush_one()
        for u in tail_stash:
            filler.append(u)
        tail_stash = []
        fire_evac_b()

        # drain whatever filler remains (tail output projections)
        while filler:
            run_one()

    nc.compile()
    return nc


_NC_CACHE = {}


def _get_nc():
    if "nc" not in _NC_CACHE:
        _NC_CACHE["nc"] = _build_nc()
    return _NC_CACHE["nc"]


def _host_prep(x, wq, wk, wv, wo, token_positions):
    head_perm = np.concatenate([np.arange(0, DK, 2), np.arange(1, DK, 2)])
    pos = np.asarray(token_positions).astype(np.float32)
    half = np.arange(0, DK, 2, dtype=np.float32) / DK
    inv_freq = THETA ** (-half)
    ang = pos[:, None] * inv_freq[None, :]        # [S, 32]
    cosT = np.cos(ang).T.astype(np.float32)       # [32, S]
    sinT = np.sin(ang).T.astype(np.float32)
    c128 = np.tile(cosT, (4, 1)).astype(bf16)     # [128, S]
    s128 = np.concatenate([-sinT, sinT, -sinT, sinT], 0).astype(bf16)

    kp = np.arange(128)[:, None, None]
    jj = np.arange(4)[None, :, None]
    qf = np.arange(QT)[None, None, :]
    maskd = (qf >= kp + 128 * jj).astype(bf16)    # [128, 4, QT]

    def chunk3(arr, nchunk):
        # [nchunk*128, F] -> [128, nchunk, F] (SBUF-native layout)
        f = arr.shape[1]
        return np.ascontiguousarray(
            arr.reshape(nchunk, 128, f).transpose(1, 0, 2))

    def prep_qk(w, g):
        rows = w.reshape(H, DK, D)[g * HG:(g + 1) * HG][:, head_perm]
        return np.ascontiguousarray(rows.reshape(HG * DK, D).T).astype(bf16)

    def prep_v(w, g):
        rows = w.reshape(H, DK, D)[g * HG:(g + 1) * HG]
        return np.ascontiguousarray(rows.reshape(HG * DK, D).T).astype(bf16)

    common = {"c128": c128, "s128": s128, "maskd": maskd}
    in_maps = []
    for c in range(NCORES):
        b, g = c // NG, c % NG
        m = dict(common)
        # [NQT, 128, NDC, QT]: token-major quarters, contiguous per partition
        m["xT"] = np.ascontiguousarray(
            x[b].T.astype(bf16).reshape(NDC, 128, NQT, QT)
            .transpose(2, 1, 0, 3))
        def pairmajor(arr):
            # [D, 512] -> [NJT, 128, NDC, 128]
            return np.ascontiguousarray(
                arr.reshape(NDC, 128, NJT, 128).transpose(2, 1, 0, 3))
        m["wqT"] = pairmajor(prep_qk(wq, g))
        m["wkT"] = pairmajor(prep_qk(wk, g))
        m["wvT"] = chunk3(prep_v(wv, g), NDC)
        m["woT"] = chunk3(
            np.ascontiguousarray(wo[:, g * HG * DK:(g + 1) * HG * DK].T
                                 ).astype(bf16), NJT)
        in_maps.append(m)
    return in_maps


def kernel(x, wq, wk, wv, wo, token_positions, _trace=False):
    x = np.asarray(x, dtype=np.float32)
    in_maps = _host_prep(x, wq, wk, wv, wo, token_positions)
    nc = _get_nc()
    res = run_bass_kernel_spmd(nc, in_maps, core_ids=list(range(NCORES)),
                               trace=_trace)
    out = np.zeros((B, S, D), np.float32)
    for b in range(B):
        # outT: [128, NMT, S] bf16 partials; row d of out.T is [mt*128+p]
        acc = (res.results[2 * b]["outT"].astype(np.float32) +
               res.results[2 * b + 1]["outT"].astype(np.float32))
        full = acc.transpose(1, 0, 2).reshape(D, S)
        out[b] = full.T
    if _trace:
        kernel.last_results = res
    return out


# revision 55
# speedup vs baseline: 1.0041x; 1.0041x over previous
"""Trainium2 Bass kernel: causal multi-head self-attention with RoPE.

Problem: B=4, S=2048, D=1024, H=16, DK=64.  out = softmax(causal(qk^T/8)) v @ wo^T
with q,k RoPE-rotated.

Sharding: 8 cores = (batch b in 0..3) x (head-group g in 0..1, 8 heads each).
Each core computes its batch's QKV for its 8 heads, causal attention, and a
partial output projection; the host sums the two group-partials per batch.

Schedule: qc-major block order (all pairs at qc=0, then qc=1, ...) so the
output projection for each qc unlocks at the 25/50/75% marks and filler work
is spread across the whole kernel instead of drying up after pair 1.  The
per-chunk chain scores->exp->mask->AV is pipelined with the AV emitted three
chunks behind its scores and carried ACROSS block boundaries (a global pend
queue), so the scalar-engine exp hides behind PE work with no clock model.
Filler (v/q/k projections, rope, output projection) is paced by a quota:
after every chunk the emitter drains filler units until the global
done-count matches the chunk fraction; the rope of the next two blocks is
pre-pulled mid-block so block starts never wait on the rope DMA+DVE chain.

Key perf mechanisms found by tracing:
- softmax denominators: the ones-column rides the AV stationary into PSUM
  row 64; the 1/den broadcast is built WITHOUT any DMA roundtrip (the old
  [1,512]->[64,512] broadcast DMA took 6-11us serializing on one DMA
  engine): the ov bank is evacuated by a single ScalarE copy (ACT reads
  PSUM fast and is ~50% idle), then DVE stream_shuffle broadcasts the
  denominator row to a base-0 window, reciprocal_approx_fast, and a
  quadrant-copy shuffle build [64,512] of 1/den.  DVE compute ops cannot
  cross base partitions (verified: garbage results), shuffles can.
- diagonal trim: the two upper diagonal k-chunks of every block have
  q<256 fully masked, so scores/exp/mask/AV compute only the [256:512)
  q-window (the two heads' trimmed windows stay in separate PSUM banks --
  two concurrent row-tiled matmuls writing one bank locks the device).
- input DMAs are ~20 large transfers with 2-8KB contiguous partition lines
  (DMA rate is line-count-bound; dma_start issue is ~0.7us of queue time),
  in first-use order across sync/scalar/gpsimd queues; q/k weights are
  staged pair-major so pair 0's 0.25MB slice lands first and the first
  projection starts ~10us after the framework preamble.
- the tail output projection is split: jc 0-2 accumulate to SBUF while the
  last block runs; only 8 jc=3 matmuls wait on the final evac chain, and a
  reserved stash of outproj units keeps the PE warm (HAM at 2.4GHz) and
  busy under that chain.
"""
import os
import sys

for _p in ("/opt/trn_rl_repo", "/root/.axon_site/_ro/trn_rl_repo"):
    if os.path.isdir(_p) and _p not in sys.path:
        sys.path.insert(0, _p)

import numpy as np
import ml_dtypes

import concourse.bass as bass
import concourse.mybir as mybir
import concourse.tile as tile
from concourse import bacc
from concourse.bass_utils import run_bass_kernel_spmd

B, S, D, H = 4, 2048, 1024, 16
DK = D // H          # 64
HG = 8               # heads per group
NG = 2               # head groups (cores per batch)
THETA = 10000.0
NCORES = 8

BF16 = mybir.dt.bfloat16
F32 = mybir.dt.float32
bf16 = ml_dtypes.bfloat16

QT = 512             # q tile width (free dim)
NQT = S // QT        # 4
NKT = S // 128       # 16 k chunks
NJT = HG * DK // 128  # 4 j-tiles (head pairs)
NDC = D // 128       # 8 d chunks
NMT = D // 128       # 8 output m tiles
NCHUNK = sum(4 * qc + 4 for qc in range(NQT)) * NJT   # 160


def _build_nc():
    from collections import deque

    nc = bacc.Bacc("TRN2", target_bir_lowering=False, debug=False)
    # DRAM layouts are SBUF-native [128, chunk, free] so input DMAs are a few
    # large 3D-AP transfers instead of ~100 small ones.
    # token-major x so every DMA slice is contiguous per partition row
    xT = nc.dram_tensor("xT", [NQT, 128, NDC, QT], BF16,
                        kind="ExternalInput").ap()
    # pair-major q/k weights: pair 0's 0.25MB slice lands early so the
    # first projections + rope start ~10us sooner
    wqT = nc.dram_tensor("wqT", [NJT, 128, NDC, 128], BF16,
                         kind="ExternalInput").ap()
    wkT = nc.dram_tensor("wkT", [NJT, 128, NDC, 128], BF16,
                         kind="ExternalInput").ap()
    wvT = nc.dram_tensor("wvT", [128, NDC, HG * DK], BF16,
                         kind="ExternalInput").ap()
    woT = nc.dram_tensor("woT", [128, NJT, D], BF16, kind="ExternalInput").ap()
    c128 = nc.dram_tensor("c128", [128, S], BF16, kind="ExternalInput").ap()
    s128 = nc.dram_tensor("s128", [128, S], BF16, kind="ExternalInput").ap()
    maskd = nc.dram_tensor("maskd", [128, 4, QT], BF16, kind="ExternalInput").ap()
    outT = nc.dram_tensor("outT", [128, NMT, S], BF16, kind="ExternalOutput").ap()

    from contextlib import ExitStack
    with tile.TileContext(nc) as tc, ExitStack() as stk:
        pp = stk.enter_context(tc.tile_pool(name="persist", bufs=1))
        ep = stk.enter_context(tc.tile_pool(name="epool", bufs=6))
        sp = stk.enter_context(tc.tile_pool(name="smalls", bufs=2))
        qw = stk.enter_context(tc.tile_pool(name="qkvwork", bufs=2))
        ps_st = stk.enter_context(
            tc.tile_pool(name="ps_st", bufs=2, space="PSUM"))
        ps_ov = stk.enter_context(
            tc.tile_pool(name="ps_ov", bufs=2, space="PSUM"))
        ps_qkv = stk.enter_context(
            tc.tile_pool(name="ps_qkv", bufs=2, space="PSUM"))

        # ---------------- persistent tiles ----------------
        wo_sb = pp.tile([128, NJT, D], BF16)
        m_sb = pp.tile([128, 4, QT], BF16)
        qrot = pp.tile([128, NJT, S], BF16)
        krot = pp.tile([128, NJT, S], BF16)
        v_aug = pp.tile([128, NKT, HG, 66], BF16)
        a_t = pp.tile([128, NJT, S], BF16)
        # token-quarter-major x: [:, t, dc, :] slices are contiguous per
        # partition on both the DRAM and SBUF side (8KB lines -> fast DMA)
        xT_sb = pp.tile([128, NQT, NDC, QT], BF16)
        wq_sb = pp.tile([128, NJT, NDC, 128], BF16)
        wk_sb = pp.tile([128, NJT, NDC, 128], BF16)
        wv_sb = pp.tile([128, NDC, HG * DK], BF16)
        c_sb = pp.tile([128, S], BF16)
        s_sb = pp.tile([128, S], BF16)

        nc.gpsimd.memset(v_aug[:, :, :, 64:65], 1.0)

        # ------- input DMAs: few big transfers, ordered by first use -------
        # sync carries dc 0-3, scalar dc 4-7 (scalar queue is free until the
        # first exp); gpsimd carries only small latency-critical transfers.
        nc.gpsimd.dma_start(c_sb[:], c128[:])
        nc.gpsimd.dma_start(s_sb[:], s128[:])
        nc.gpsimd.dma_start(m_sb[:], maskd[:])
        # few large transfers; per-partition lines are 4-8KB contiguous.
        # wq/x-t0 interleave across both queues so the first projection
        # matmuls (dc-pair granularity) unblock as early as possible.
        nc.scalar.dma_start(wq_sb[:, 0], wqT[0])
        nc.scalar.dma_start(wk_sb[:, 0], wkT[0])
        nc.sync.dma_start(xT_sb[:, 0, 0:4, :], xT[0, :, 0:4, :])
        nc.sync.dma_start(xT_sb[:, 0, 4:8, :], xT[0, :, 4:8, :])
        nc.scalar.dma_start(wq_sb[:, 1], wqT[1])
        nc.scalar.dma_start(wk_sb[:, 1], wkT[1])
        nc.sync.dma_start(xT_sb[:, 1, 0:4, :], xT[1, :, 0:4, :])
        nc.sync.dma_start(xT_sb[:, 1, 4:8, :], xT[1, :, 4:8, :])
        nc.scalar.dma_start(wq_sb[:, 2], wqT[2])
        nc.scalar.dma_start(wk_sb[:, 2], wkT[2])
        nc.scalar.dma_start(wq_sb[:, 3], wqT[3])
        nc.scalar.dma_start(wk_sb[:, 3], wkT[3])
        nc.sync.dma_start(wv_sb[:], wvT[:])
        nc.scalar.dma_start(xT_sb[:, 2], xT[2])
        nc.sync.dma_start(xT_sb[:, 3], xT[3])
        nc.scalar.dma_start(wo_sb[:], woT[:])

        # ---------------- emit helpers ----------------
        def proj_mms(pair, w_sb, ps, tn, dlo, dhi):
            for dc in range(dlo, dhi):
                nc.tensor.matmul(
                    ps[:],
                    w_sb[:, pair, dc, :],
                    xT_sb[:, tn, dc, :],
                    start=(dc == 0), stop=(dc == NDC - 1))

        def rope_dma(pair, name, pre, hf, dma_eng=None):
            # stage the 32-block partition swap of pre (DMA only, no DVE);
            # issued ~8 filler units before rope_mul so the DVE never
            # head-of-line blocks on this transfer.
            swp = qw.tile([128, 1024], BF16, tag="swp" + name,
                          name=f"swp{name}{pair}{hf}")
            for a in range(4):
                lo, sw = 32 * a, 32 * (a ^ 1)
                (dma_eng or nc.gpsimd).dma_start(
                    swp[lo:lo + 32, :], pre[sw:sw + 32, :])
            return swp

        def rope_mul(pair, pre, swp, dst, hf):
            cs = slice(hf * 1024, (hf + 1) * 1024)
            nc.vector.tensor_mul(dst[:, pair, cs], pre[:], c_sb[:, cs])
            nc.vector.tensor_mul(swp[:], swp[:], s_sb[:, cs])
            nc.vector.tensor_add(dst[:, pair, cs], dst[:, pair, cs], swp[:])

        def emit_scores(pair, qc, kc):
            # one k-chunk, both heads of the pair packed into one st tile.
            # Full chunks: st[:, 0:512] = head 0, st[:, 512:1024] = head 1.
            # Upper-diagonal chunks (par>=2) have q < 256 fully masked, so
            # only the q-window [256:512) is computed, packed as
            # st[:, 0:256] = head 0, st[:, 256:512] = head 1.
            par = kc - 4 * qc
            st = ps_st.tile([128, 2 * QT], F32, tag="st",
                            name=f"st_{pair}{qc}{kc}")
            w = 256 if par >= 2 else QT
            qlo = qc * QT + (QT - w)
            for h01 in range(2):
                lo = 64 * h01
                nc.tensor.matmul(
                    st[:, h01 * QT:h01 * QT + w],
                    krot[lo:lo + 64, pair, kc * 128:(kc + 1) * 128],
                    qrot[lo:lo + 64, pair, qlo:(qc + 1) * QT],
                    start=True, stop=True,
                    tile_position=(lo, 0))
            return st

        def emit_exp(pair, qc, kc, st):
            par = kc - 4 * qc
            e = ep.tile([128, 2 * QT], BF16, tag="e", bufs=5,
                        name=f"e{pair}{qc}{kc}")
            w = 256 if par >= 2 else QT
            if w == QT:
                nc.scalar.activation(
                    e[:], st[:], mybir.ActivationFunctionType.Exp,
                    scale=0.125)
            else:
                # one strided ACT op covering both heads' trimmed windows
                ev = e[:].rearrange("p (h x) -> p h x", h=2)[:, :, 0:w]
                sv = st[:].rearrange("p (h x) -> p h x", h=2)[:, :, 0:w]
                nc.scalar.activation(
                    ev, sv, mybir.ActivationFunctionType.Exp, scale=0.125)
            if par >= 0:        # diagonal band: mask both heads' windows
                for h01 in range(2):
                    nc.vector.tensor_mul(
                        e[:, h01 * QT:h01 * QT + w],
                        e[:, h01 * QT:h01 * QT + w],
                        m_sb[:, par, QT - w:QT])
            return e

        def emit_av(pair, qc, kc, e, ov0, ov1, last):
            par = kc - 4 * qc
            w = 256 if par >= 2 else QT
            for h01, ov in ((0, ov0), (1, ov1)):
                nc.tensor.matmul(
                    ov[0:65, QT - w:QT],
                    v_aug[:, kc, 2 * pair + h01, 0:65],
                    e[:, h01 * QT:h01 * QT + w],
                    start=(kc == 0),
                    stop=last, skip_group_check=True)

        def emit_evac_a(pair, qc, ov0, ov1):
            """Free the ov banks and build the 1/den broadcast entirely on
            the DVE: broadcast-shuffle the PSUM denominator row down to a
            base-0 window, reciprocal there, quadrant-copy to 64 rows.  No
            DMA roundtrips, so the deferred normalize (emit_evac_b) never
            head-of-line blocks the DVE FIFO."""
            ous = []
            for h01, ov in ((0, ov0), (1, ov1)):
                # single scalar-engine copy (incl. the denominator row) is
                # the only reader of the ov bank, so the bank frees ~1.5us
                # after the last AV regardless of DVE queue depth.  The DVE
                # reciprocal-broadcast chain is deferred to emit_evac_b so
                # it never sits ahead of rope/mask work at block boundaries.
                ou = ep.tile([96, QT], F32, tag="ou", bufs=3,
                             name=f"ou{pair}{qc}{h01}")
                nc.scalar.copy(ou[0:65, :], ov[0:65, :])
                ous.append(ou)
            return ous, None

        def emit_evac_b(pair, qc, ous, rbs):
            rbs = []
            for h01 in range(2):
                den = sp.tile([32, QT], F32, tag="den", bufs=2,
                              name=f"den{pair}{qc}{h01}")
                nc.vector.stream_shuffle(den[:], ous[h01][64:96, :],
                                         mask=[0] * 32)
                rb = sp.tile([64, QT], F32, tag="rb", bufs=4,
                             name=f"rb{pair}{qc}{h01}")
                nc.vector.reciprocal_approx_fast(rb[0:32, :], den[:])
                nc.vector.stream_shuffle(rb[32:64, :], rb[0:32, :],
                                         mask=list(range(32)))
                rbs.append(rb)
            nc.vector.tensor_mul(
                a_t[0:64, pair, qc * QT:(qc + 1) * QT],
                ous[0][0:64, :], rbs[0][:])
            an = sp.tile([64, QT], BF16, tag="an", bufs=3,
                         name=f"an{pair}{qc}")
            nc.vector.tensor_mul(an[:], ous[1][0:64, :], rbs[1][:])
            nc.gpsimd.dma_start(
                a_t[64:128, pair, qc * QT:(qc + 1) * QT], an[:])

        # ---------------- filler unit machinery (no clock model) ----------
        filler = deque()     # items: (label, fn)
        emitted = set()
        state = {"done": 0}
        TOTAL_UNITS = (NKT * 4) + (NJT * 2 * 20) + (3 * NMT * 2) + 24  # 296

        def run_one():
            lab, fn = filler.popleft()
            fn()
            state["done"] += 1
            if lab is not None:
                emitted.add(lab)

        def drain_until(lab):
            while lab not in emitted:
                assert filler, f"filler ran dry before {lab}"
                run_one()

        def pace(frac):
            tgt = frac * TOTAL_UNITS
            while filler and state["done"] < tgt:
                run_one()

        # --- v projection units: 16 token-tiles, 4 chunks of 2 dc each ---
        def v_units(tlo, thi):
            for tt in range(tlo, thi):
                cell = {}

                def mk(tt, dlo, dhi, last):
                    def fn():
                        if dlo == 0:
                            cell["ps"] = ps_qkv.tile(
                                [128, QT], F32, tag="qv", name=f"psv{tt}")
                        ps = cell["ps"]
                        for dc in range(dlo, dhi):
                            nc.tensor.matmul(
                                ps[:],
                                xT_sb[:, tt // 4, dc,
                                      (tt % 4) * 128:(tt % 4 + 1) * 128],
                                wv_sb[:, dc, :],
                                start=(dc == 0), stop=(dc == NDC - 1))
                        if last:
                            nc.vector.tensor_copy(
                                v_aug[:, tt, :, 0:64],
                                ps[:].rearrange("p (h d) -> p h d", h=HG))
                    return fn
                for ci in range(4):
                    dlo, dhi = 2 * ci, 2 * ci + 2
                    lab = ("v", tt) if ci == 3 else None
                    yield (lab, mk(tt, dlo, dhi, ci == 3))

        # --- q/k projection + rope units for one (pair, half) ---
        def proj_half_units(pair, hf, swp_eng=None):
            preq = qw.tile([128, 1024], BF16, tag="preq",
                           name=f"preq{pair}{hf}")
            prek = qw.tile([128, 1024], BF16, tag="prek",
                           name=f"prek{pair}{hf}")
            swps = {}

            def mkp(w_sb, pre, tn, dlo, dhi, last, cell):
                def fn():
                    if dlo == 0:
                        cell["ps"] = ps_qkv.tile(
                            [128, QT], F32, tag="qv",
                            name=f"psp{pair}{tn}")
                    ps = cell["ps"]
                    proj_mms(pair, w_sb, ps, tn, dlo, dhi)
                    if last:
                        nc.vector.tensor_copy(
                            pre[:, (tn - 2 * hf) * QT:(tn - 2 * hf + 1) * QT],
                            ps[:])
                return fn

            def mkd(name, pre):
                def fn():
                    swps[name] = rope_dma(pair, name, pre, hf,
                                          dma_eng=swp_eng)
                return fn

            def mkm(name, pre, dst):
                def fn():
                    rope_mul(pair, pre, swps[name], dst, hf)
                return fn

            # tn-interleaved (q-tn0, k-tn0, q-tn1, k-tn1) to match the DMA
            # arrival order in the head phase (x-t0 + wq, then wk, then x-t1)
            for tn_i in range(2):
                for name, w_sb, pre in (("q", wq_sb, preq),
                                        ("k", wk_sb, prek)):
                    tn = 2 * hf + tn_i
                    cell = {}
                    for ci in range(4):
                        yield (None, mkp(w_sb, pre, tn, 2 * ci,
                                         2 * ci + 2, ci == 3, cell))
            for name, pre in (("q", preq), ("k", prek)):
                yield (None, mkd(name, pre))
            for name, pre, dst in (("q", preq, qrot), ("k", prek, krot)):
                yield (("rope" + name, pair, hf), mkm(name, pre, dst))

        # --- output projection units for one qc: 8 mt, 2 chunks of 2 jc ---
        def outproj_units(qc):
            for mt in range(NMT):
                cell = {}

                def mk(qc, mt, jlo, jhi, last):
                    def fn():
                        if jlo == 0:
                            cell["ps"] = ps_qkv.tile(
                                [128, QT], F32, tag="qv", name=f"op{qc}{mt}")
                        op = cell["ps"]
                        for jc in range(jlo, jhi):
                            nc.tensor.matmul(
                                op[:],
                                wo_sb[:, jc, mt * 128:(mt + 1) * 128],
                                a_t[:, jc, qc * QT:(qc + 1) * QT],
                                start=(jc == 0), stop=(jc == NJT - 1))
                        if last:
                            ot = sp.tile([128, QT], BF16, tag="ot", bufs=3,
                                         name=f"ot{qc}{mt}")
                            nc.vector.tensor_copy(ot[:], op[:])
                            nc.sync.dma_start(
                                outT[:, mt, qc * QT:(qc + 1) * QT],
                                ot[:])
                    return fn
                for ci in range(2):
                    yield (None, mk(qc, mt, 2 * ci, 2 * ci + 2, ci == 1))

        # final round's outproj is split: jc 0-2 accumulate to SBUF while the
        # last block still runs; only the 8 jc=3 matmuls wait on the final
        # evac chain, shrinking the kernel tail from ~8us to ~2.5us.
        ota_hold = {}

        def outproj_a_units(qc):
            for mt in range(NMT):
                def mk(qc, mt):
                    def fn():
                        op = ps_qkv.tile([128, QT], F32, tag="qv",
                                         name=f"opA{qc}{mt}")
                        for jc in range(3):
                            nc.tensor.matmul(
                                op[:],
                                wo_sb[:, jc, mt * 128:(mt + 1) * 128],
                                a_t[:, jc, qc * QT:(qc + 1) * QT],
                                start=(jc == 0), stop=(jc == 2))
                        ota = sp.tile([128, QT], F32, tag="ota", bufs=8,
                                      name=f"ota{mt}")
                        nc.vector.tensor_copy(ota[:], op[:])
                        ota_hold[mt] = ota
                    return fn
                yield (None, mk(qc, mt))

        def outproj_b_units(qc):
            for mt in range(NMT):
                def mk(qc, mt):
                    def fn():
                        op = ps_qkv.tile([128, QT], F32, tag="qv",
                                         name=f"opB{qc}{mt}")
                        nc.tensor.matmul(
                            op[:],
                            wo_sb[:, 3, mt * 128:(mt + 1) * 128],
                            a_t[:, 3, qc * QT:(qc + 1) * QT],
                            start=True, stop=True)
                        ot = sp.tile([128, QT], BF16, tag="ot", bufs=3,
                                     name=f"otB{qc}{mt}")
                        nc.vector.tensor_add(ot[:], op[:], ota_hold[mt][:])
                        nc.sync.dma_start(
                            outT[:, mt, qc * QT:(qc + 1) * QT], ot[:])
                    return fn
                yield (None, mk(qc, mt))

        # ------ phase 0: pair-0 first-half q/k projections + rope up front
        for lab, fn in proj_half_units(0, 0, swp_eng=nc.gpsimd):
            fn()
            state["done"] += 1
            if lab is not None:
                emitted.add(lab)

        # filler deque ordered so qc-major forced drains pull just-in-time:
        # round qc0 needs v tt0-3 + all pairs' half-0; round qc1 needs v
        # tt4-7; round qc2 needs half-1 rope + v tt8-11; round qc3 the rest.
        for u in v_units(0, 4):
            filler.append(u)
        for pair in (1, 2, 3):
            for u in proj_half_units(pair, 0):
                filler.append(u)
        for u in v_units(4, 8):
            filler.append(u)
        for u in proj_half_units(0, 1):
            filler.append(u)
        for u in v_units(8, 12):
            filler.append(u)
        for u in proj_half_units(1, 1):
            filler.append(u)
        for u in proj_half_units(2, 1):
            filler.append(u)
        for u in proj_half_units(3, 1):
            filler.append(u)
        for u in v_units(12, 16):
            filler.append(u)
        # outproj units are appended as each qc round's last evac_b fires

        # -------- main attention loop (qc-major, cross-block pipeline) ----
        # The AV+evac tail of each block is interleaved with the next
        # block's scores via a global pend queue, so the PE never drains
        # while the last exps of a block finish.
        hold = {"b": None, "age": 0}  # (pair, qc, ous, rbs) + chunks waited

        def fire_evac_b():
            bpair, bqc, ous, rbs = hold["b"]
            emit_evac_b(bpair, bqc, ous, rbs)
            hold["b"] = None
            if bqc == NQT - 1 and bpair == NJT - 2:
                for u in outproj_a_units(bqc):
                    filler.append(u)
            if bpair == NJT - 1:
                if bqc == NQT - 1:
                    gen = list(outproj_b_units(bqc))
                elif bqc == NQT - 2:
                    # hold back the last 6 mt so the PE stays warm (and
                    # busy) during the final block's evac chain
                    units = list(outproj_units(bqc))
                    tail_stash.extend(units[-16:])
                    gen = units[:-16]
                else:
                    gen = list(outproj_units(bqc))
                for u in gen:
                    filler.append(u)

        blocks = [(qc, pair) for qc in range(NQT) for pair in range(NJT)]
        pend = deque()   # (pair, qc, kc, e, ov0, ov1, last)
        tail_stash = []

        def flush_one():
            fpair, fqc, kc, e, ov0, ov1, last = pend.popleft()
            if fpair == 0:
                drain_until(("v", kc))
            emit_av(fpair, fqc, kc, e, ov0, ov1, last)
            if last:
                if hold["b"] is not None:   # short blocks: fire before reuse
                    fire_evac_b()
                hold["b"] = (fpair, fqc) + emit_evac_a(fpair, fqc, ov0, ov1)
                hold["age"] = 0

        gchunk = 0
        for bi, (qc, pair) in enumerate(blocks):
            ngrp = 4 * qc + 4      # k chunks covering causal range
            ov0 = ps_ov.tile([96, QT], F32, tag="ov", name=f"ov0_{pair}{qc}")
            ov1 = ps_ov.tile([96, QT], F32, tag="ov", name=f"ov1_{pair}{qc}")
            drain_until(("ropeq", pair, qc // 2))
            drain_until(("ropek", pair, 0))
            for kc in range(ngrp):
                if kc >= 8:
                    drain_until(("ropek", pair, 1))
                if pair == 0:
                    drain_until(("v", min(kc + 2, ngrp - 1)))
                st = emit_scores(pair, qc, kc)
                e = emit_exp(pair, qc, kc, st)
                pend.append((pair, qc, kc, e, ov0, ov1, kc == ngrp - 1))
                # fire the deferred normalize only once its broadcast DMA
                # chain (~4 chunks) has surely landed, so the DVE FIFO never
                # head-of-line blocks the masks queued behind it
                hold["age"] += 1
                if hold["b"] is not None and hold["age"] >= 6:
                    fire_evac_b()
                gchunk += 1
                pace((gchunk - 0.5) / NCHUNK)
                if len(pend) > 3:
                    flush_one()
                pace(gchunk / NCHUNK)
                # the final block is exp-bound once regular filler runs dry;
                # feed half the reserved stash through it, keep the rest for
                # the evac-chain tail
                if bi == len(blocks) - 1 and kc % 2 == 0 and                         len(tail_stash) > 8 and not filler:
                    filler.append(tail_stash.pop(0))
                    run_one()
                if kc in (1, 2) and bi + kc < len(blocks):
                    # pre-pull the rope (and projections) of the next TWO
                    # blocks so a block start never waits on the rope
                    # DMA+DVE chain, even across the short qc=0 blocks
                    nqc, npair = blocks[bi + kc]
                    drain_until(("ropeq", npair, nqc // 2))
                    drain_until(("ropek", npair, 0))
                    if nqc >= 2:
                        drain_until(("ropek", npair, 1))

        # drain the pipeline: remaining AVs with reserved filler interleaved;
        # the bulk of the reserve runs after the final evac_a so the PE stays
        # warm (and useful) under the final DVE evac chain
        while pend:
            if tail_stash:
                filler.append(tail_stash.pop(0))
            if filler:
                run_one()
            flush_one()
        for u in tail_stash:
            filler.append(u)
        tail_stash = []
        fire_evac_b()

        # drain whatever filler remains (tail output projections)
        while filler:
            run_one()

    nc.compile()
    return nc


_NC_CACHE = {}


def _get_nc():
    if "nc" not in _NC_CACHE:
        _NC_CACHE["nc"] = _build_nc()
    return _NC_CACHE["nc"]


def _host_prep(x, wq, wk, wv, wo, token_positions):
    head_perm = np.concatenate([np.arange(0, DK, 2), np.arange(1, DK, 2)])
    pos = np.asarray(token_positions).astype(np.float32)
    half = np.arange(0, DK, 2, dtype=np.float32) / DK
    inv_freq = THETA ** (-half)
    ang = pos[:, None] * inv_freq[None, :]        # [S, 32]
    cosT = np.cos(ang).T.astype(np.float32)       # [32, S]
    sinT = np.sin(ang).T.astype(np.float32)
    c128 = np.tile(cosT, (4, 1)).astype(bf16)     # [128, S]
    s128 = np.concatenate([-sinT, sinT, -sinT, sinT], 0).astype(bf16)

    kp = np.arange(128)[:, None, None]
    jj = np.arange(4)[None, :, None]
    qf = np.arange(QT)[None, None, :]
    maskd = (qf >= kp + 128 * jj).astype(bf16)    # [128, 4, QT]

    def chunk3(arr, nchunk):
        # [nchunk*128, F] -> [128, nchunk, F] (SBUF-native layout)
        f = arr.shape[1]
        return np.ascontiguousarray(
            arr.reshape(nchunk, 128, f).transpose(1, 0, 2))

    def prep_qk(w, g):
        rows = w.reshape(H, DK, D)[g * HG:(g + 1) * HG][:, head_perm]
        return np.ascontiguousarray(rows.reshape(HG * DK, D).T).astype(bf16)

    def prep_v(w, g):
        rows = w.reshape(H, DK, D)[g * HG:(g + 1) * HG]
        return np.ascontiguousarray(rows.reshape(HG * DK, D).T).astype(bf16)

    common = {"c128": c128, "s128": s128, "maskd": maskd}
    in_maps = []
    for c in range(NCORES):
        b, g = c // NG, c % NG
        m = dict(common)
        # [NQT, 128, NDC, QT]: token-major quarters, contiguous per partition
        m["xT"] = np.ascontiguousarray(
            x[b].T.astype(bf16).reshape(NDC, 128, NQT, QT)
            .transpose(2, 1, 0, 3))
        def pairmajor(arr):
            # [D, 512] -> [NJT, 128, NDC, 128]
            return np.ascontiguousarray(
                arr.reshape(NDC, 128, NJT, 128).transpose(2, 1, 0, 3))
        m["wqT"] = pairmajor(prep_qk(wq, g))
        m["wkT"] = pairmajor(prep_qk(wk, g))
        m["wvT"] = chunk3(prep_v(wv, g), NDC)
        m["woT"] = chunk3(
            np.ascontiguousarray(wo[:, g * HG * DK:(g + 1) * HG * DK].T
                                 ).astype(bf16), NJT)
        in_maps.append(m)
    return in_maps


def kernel(x, wq, wk, wv, wo, token_positions, _trace=False):
    x = np.asarray(x, dtype=np.float32)
    in_maps = _host_prep(x, wq, wk, wv, wo, token_positions)
    nc = _get_nc()
    res = run_bass_kernel_spmd(nc, in_maps, core_ids=list(range(NCORES)),
                               trace=_trace)
    out = np.zeros((B, S, D), np.float32)
    for b in range(B):
        # outT: [128, NMT, S] bf16 partials; row d of out.T is [mt*128+p]
        acc = (res.results[2 * b]["outT"].astype(np.float32) +
               res.results[2 * b + 1]["outT"].astype(np.float32))
        full = acc.transpose(1, 0, 2).reshape(D, S)
        out[b] = full.T
    if _trace:
        kernel.last_results = res
    return out


# revision 56
# speedup vs baseline: 1.0089x; 1.0047x over previous
"""Trainium2 Bass kernel: causal multi-head self-attention with RoPE.

Problem: B=4, S=2048, D=1024, H=16, DK=64.  out = softmax(causal(qk^T/8)) v @ wo^T
with q,k RoPE-rotated.

Sharding: 8 cores = (batch b in 0..3) x (head-group g in 0..1, 8 heads each).
Each core computes its batch's QKV for its 8 heads, causal attention, and a
partial output projection; the host sums the two group-partials per batch.

Schedule: qc-major block order (all pairs at qc=0, then qc=1, ...) so the
output projection for each qc unlocks at the 25/50/75% marks and filler work
is spread across the whole kernel instead of drying up after pair 1.  The
per-chunk chain scores->exp->mask->AV is pipelined with the AV emitted three
chunks behind its scores and carried ACROSS block boundaries (a global pend
queue), so the scalar-engine exp hides behind PE work with no clock model.
Filler (v/q/k projections, rope, output projection) is paced by a quota:
after every chunk the emitter drains filler units until the global
done-count matches the chunk fraction; the rope of the next two blocks is
pre-pulled mid-block so block starts never wait on the rope DMA+DVE chain.

Key perf mechanisms found by tracing:
- softmax denominators: the ones-column rides the AV stationary into PSUM
  row 64; the 1/den broadcast is built WITHOUT any DMA roundtrip (the old
  [1,512]->[64,512] broadcast DMA took 6-11us serializing on one DMA
  engine): the ov bank is evacuated by a single ScalarE copy (ACT reads
  PSUM fast and is ~50% idle), then DVE stream_shuffle broadcasts the
  denominator row to a base-0 window, reciprocal_approx_fast, and a
  quadrant-copy shuffle build [64,512] of 1/den.  DVE compute ops cannot
  cross base partitions (verified: garbage results), shuffles can.
- diagonal trim: the two upper diagonal k-chunks of every block have
  q<256 fully masked, so scores/exp/mask/AV compute only the [256:512)
  q-window (the two heads' trimmed windows stay in separate PSUM banks --
  two concurrent row-tiled matmuls writing one bank locks the device).
- input DMAs are ~20 large transfers with 2-8KB contiguous partition lines
  (DMA rate is line-count-bound; dma_start issue is ~0.7us of queue time),
  in first-use order across sync/scalar/gpsimd queues; q/k weights are
  staged pair-major so pair 0's 0.25MB slice lands first and the first
  projection starts ~10us after the framework preamble.
- the tail output projection is split: jc 0-2 accumulate to SBUF while the
  last block runs; only 8 jc=3 matmuls wait on the final evac chain, and a
  reserved stash of outproj units keeps the PE warm (HAM at 2.4GHz) and
  busy under that chain.
"""
import os
import sys

for _p in ("/opt/trn_rl_repo", "/root/.axon_site/_ro/trn_rl_repo"):
    if os.path.isdir(_p) and _p not in sys.path:
        sys.path.insert(0, _p)

import numpy as np
import ml_dtypes

import concourse.bass as bass
import concourse.mybir as mybir
import concourse.tile as tile
from concourse import bacc
from concourse.bass_utils import run_bass_kernel_spmd

B, S, D, H = 4, 2048, 1024, 16
DK = D // H          # 64
HG = 8               # heads per group
NG = 2               # head groups (cores per batch)
THETA = 10000.0
NCORES = 8

BF16 = mybir.dt.bfloat16
F32 = mybir.dt.float32
bf16 = ml_dtypes.bfloat16

QT = 512             # q tile width (free dim)
NQT = S // QT        # 4
NKT = S // 128       # 16 k chunks
NJT = HG * DK // 128  # 4 j-tiles (head pairs)
NDC = D // 128       # 8 d chunks
NMT = D // 128       # 8 output m tiles
NCHUNK = sum(4 * qc + 4 for qc in range(NQT)) * NJT   # 160


def _build_nc():
    from collections import deque

    nc = bacc.Bacc("TRN2", target_bir_lowering=False, debug=False)
    # DRAM layouts are SBUF-native [128, chunk, free] so input DMAs are a few
    # large 3D-AP transfers instead of ~100 small ones.
    # token-major x so every DMA slice is contiguous per partition row
    xT = nc.dram_tensor("xT", [NQT, 128, NDC, QT], BF16,
                        kind="ExternalInput").ap()
    # pair-major q/k weights: pair 0's 0.25MB slice lands early so the
    # first projections + rope start ~10us sooner
    wqT = nc.dram_tensor("wqT", [NJT, 128, NDC, 128], BF16,
                         kind="ExternalInput").ap()
    wkT = nc.dram_tensor("wkT", [NJT, 128, NDC, 128], BF16,
                         kind="ExternalInput").ap()
    wvT = nc.dram_tensor("wvT", [128, NDC, HG * DK], BF16,
                         kind="ExternalInput").ap()
    woT = nc.dram_tensor("woT", [128, NJT, D], BF16, kind="ExternalInput").ap()
    c128 = nc.dram_tensor("c128", [128, S], BF16, kind="ExternalInput").ap()
    s128 = nc.dram_tensor("s128", [128, S], BF16, kind="ExternalInput").ap()
    maskd = nc.dram_tensor("maskd", [128, 4, QT], BF16, kind="ExternalInput").ap()
    outT = nc.dram_tensor("outT", [128, NMT, S], BF16, kind="ExternalOutput").ap()

    from contextlib import ExitStack
    with tile.TileContext(nc) as tc, ExitStack() as stk:
        pp = stk.enter_context(tc.tile_pool(name="persist", bufs=1))
        ep = stk.enter_context(tc.tile_pool(name="epool", bufs=6))
        sp = stk.enter_context(tc.tile_pool(name="smalls", bufs=2))
        qw = stk.enter_context(tc.tile_pool(name="qkvwork", bufs=2))
        ps_st = stk.enter_context(
            tc.tile_pool(name="ps_st", bufs=2, space="PSUM"))
        ps_ov = stk.enter_context(
            tc.tile_pool(name="ps_ov", bufs=2, space="PSUM"))
        ps_qkv = stk.enter_context(
            tc.tile_pool(name="ps_qkv", bufs=2, space="PSUM"))

        # ---------------- persistent tiles ----------------
        wo_sb = pp.tile([128, NJT, D], BF16)
        m_sb = pp.tile([128, 4, QT], BF16)
        qrot = pp.tile([128, NJT, S], BF16)
        krot = pp.tile([128, NJT, S], BF16)
        v_aug = pp.tile([128, NKT, HG, 66], BF16)
        a_t = pp.tile([128, NJT, S], BF16)
        # token-quarter-major x: [:, t, dc, :] slices are contiguous per
        # partition on both the DRAM and SBUF side (8KB lines -> fast DMA)
        xT_sb = pp.tile([128, NQT, NDC, QT], BF16)
        wq_sb = pp.tile([128, NJT, NDC, 128], BF16)
        wk_sb = pp.tile([128, NJT, NDC, 128], BF16)
        wv_sb = pp.tile([128, NDC, HG * DK], BF16)
        c_sb = pp.tile([128, S], BF16)
        s_sb = pp.tile([128, S], BF16)

        nc.gpsimd.memset(v_aug[:, :, :, 64:65], 1.0)

        # ------- input DMAs: few big transfers, ordered by first use -------
        # sync carries dc 0-3, scalar dc 4-7 (scalar queue is free until the
        # first exp); gpsimd carries only small latency-critical transfers.
        nc.gpsimd.dma_start(c_sb[:], c128[:])
        nc.gpsimd.dma_start(s_sb[:], s128[:])
        nc.gpsimd.dma_start(m_sb[:], maskd[:])
        # few large transfers; per-partition lines are 4-8KB contiguous.
        # wq/x-t0 interleave across both queues so the first projection
        # matmuls (dc-pair granularity) unblock as early as possible.
        nc.scalar.dma_start(wq_sb[:, 0], wqT[0])
        nc.scalar.dma_start(wk_sb[:, 0], wkT[0])
        nc.sync.dma_start(xT_sb[:, 0, 0:4, :], xT[0, :, 0:4, :])
        nc.sync.dma_start(xT_sb[:, 0, 4:8, :], xT[0, :, 4:8, :])
        nc.scalar.dma_start(wq_sb[:, 1], wqT[1])
        nc.scalar.dma_start(wk_sb[:, 1], wkT[1])
        nc.sync.dma_start(wv_sb[:], wvT[:])
        nc.sync.dma_start(xT_sb[:, 1, 0:4, :], xT[1, :, 0:4, :])
        nc.sync.dma_start(xT_sb[:, 1, 4:8, :], xT[1, :, 4:8, :])
        nc.scalar.dma_start(wq_sb[:, 2], wqT[2])
        nc.scalar.dma_start(wk_sb[:, 2], wkT[2])
        nc.scalar.dma_start(wq_sb[:, 3], wqT[3])
        nc.scalar.dma_start(wk_sb[:, 3], wkT[3])
        nc.scalar.dma_start(xT_sb[:, 2], xT[2])
        nc.sync.dma_start(xT_sb[:, 3], xT[3])
        nc.scalar.dma_start(wo_sb[:], woT[:])

        # ---------------- emit helpers ----------------
        def proj_mms(pair, w_sb, ps, tn, dlo, dhi):
            for dc in range(dlo, dhi):
                nc.tensor.matmul(
                    ps[:],
                    w_sb[:, pair, dc, :],
                    xT_sb[:, tn, dc, :],
                    start=(dc == 0), stop=(dc == NDC - 1))

        def rope_dma(pair, name, pre, hf, dma_eng=None):
            # stage the 32-block partition swap of pre (DMA only, no DVE);
            # issued ~8 filler units before rope_mul so the DVE never
            # head-of-line blocks on this transfer.
            swp = qw.tile([128, 1024], BF16, tag="swp" + name,
                          name=f"swp{name}{pair}{hf}")
            for a in range(4):
                lo, sw = 32 * a, 32 * (a ^ 1)
                (dma_eng or nc.gpsimd).dma_start(
                    swp[lo:lo + 32, :], pre[sw:sw + 32, :])
            return swp

        def rope_mul(pair, pre, swp, dst, hf):
            cs = slice(hf * 1024, (hf + 1) * 1024)
            nc.vector.tensor_mul(dst[:, pair, cs], pre[:], c_sb[:, cs])
            nc.vector.tensor_mul(swp[:], swp[:], s_sb[:, cs])
            nc.vector.tensor_add(dst[:, pair, cs], dst[:, pair, cs], swp[:])

        def emit_scores(pair, qc, kc):
            # one k-chunk, both heads of the pair packed into one st tile.
            # Full chunks: st[:, 0:512] = head 0, st[:, 512:1024] = head 1.
            # Upper-diagonal chunks (par>=2) have q < 256 fully masked, so
            # only the q-window [256:512) is computed, packed as
            # st[:, 0:256] = head 0, st[:, 256:512] = head 1.
            par = kc - 4 * qc
            st = ps_st.tile([128, 2 * QT], F32, tag="st",
                            name=f"st_{pair}{qc}{kc}")
            w = 256 if par >= 2 else QT
            qlo = qc * QT + (QT - w)
            for h01 in range(2):
                lo = 64 * h01
                nc.tensor.matmul(
                    st[:, h01 * QT:h01 * QT + w],
                    krot[lo:lo + 64, pair, kc * 128:(kc + 1) * 128],
                    qrot[lo:lo + 64, pair, qlo:(qc + 1) * QT],
                    start=True, stop=True,
                    tile_position=(lo, 0))
            return st

        def emit_exp(pair, qc, kc, st):
            par = kc - 4 * qc
            e = ep.tile([128, 2 * QT], BF16, tag="e", bufs=5,
                        name=f"e{pair}{qc}{kc}")
            w = 256 if par >= 2 else QT
            if w == QT:
                nc.scalar.activation(
                    e[:], st[:], mybir.ActivationFunctionType.Exp,
                    scale=0.125)
            else:
                # one strided ACT op covering both heads' trimmed windows
                ev = e[:].rearrange("p (h x) -> p h x", h=2)[:, :, 0:w]
                sv = st[:].rearrange("p (h x) -> p h x", h=2)[:, :, 0:w]
                nc.scalar.activation(
                    ev, sv, mybir.ActivationFunctionType.Exp, scale=0.125)
            if par >= 0:        # diagonal band: mask both heads' windows
                for h01 in range(2):
                    nc.vector.tensor_mul(
                        e[:, h01 * QT:h01 * QT + w],
                        e[:, h01 * QT:h01 * QT + w],
                        m_sb[:, par, QT - w:QT])
            return e

        def emit_av(pair, qc, kc, e, ov0, ov1, last):
            par = kc - 4 * qc
            w = 256 if par >= 2 else QT
            for h01, ov in ((0, ov0), (1, ov1)):
                nc.tensor.matmul(
                    ov[0:65, QT - w:QT],
                    v_aug[:, kc, 2 * pair + h01, 0:65],
                    e[:, h01 * QT:h01 * QT + w],
                    start=(kc == 0),
                    stop=last, skip_group_check=True)

        def emit_evac_a(pair, qc, ov0, ov1):
            """Free the ov banks and build the 1/den broadcast entirely on
            the DVE: broadcast-shuffle the PSUM denominator row down to a
            base-0 window, reciprocal there, quadrant-copy to 64 rows.  No
            DMA roundtrips, so the deferred normalize (emit_evac_b) never
            head-of-line blocks the DVE FIFO."""
            ous = []
            for h01, ov in ((0, ov0), (1, ov1)):
                # single scalar-engine copy (incl. the denominator row) is
                # the only reader of the ov bank, so the bank frees ~1.5us
                # after the last AV regardless of DVE queue depth.  The DVE
                # reciprocal-broadcast chain is deferred to emit_evac_b so
                # it never sits ahead of rope/mask work at block boundaries.
                ou = ep.tile([96, QT], F32, tag="ou", bufs=3,
                             name=f"ou{pair}{qc}{h01}")
                nc.scalar.copy(ou[0:65, :], ov[0:65, :])
                ous.append(ou)
            return ous, None

        def emit_evac_b(pair, qc, ous, rbs):
            rbs = []
            for h01 in range(2):
                den = sp.tile([32, QT], F32, tag="den", bufs=2,
                              name=f"den{pair}{qc}{h01}")
                nc.vector.stream_shuffle(den[:], ous[h01][64:96, :],
                                         mask=[0] * 32)
                rb = sp.tile([64, QT], F32, tag="rb", bufs=4,
                             name=f"rb{pair}{qc}{h01}")
                nc.vector.reciprocal_approx_fast(rb[0:32, :], den[:])
                nc.vector.stream_shuffle(rb[32:64, :], rb[0:32, :],
                                         mask=list(range(32)))
                rbs.append(rb)
            nc.vector.tensor_mul(
                a_t[0:64, pair, qc * QT:(qc + 1) * QT],
                ous[0][0:64, :], rbs[0][:])
            an = sp.tile([64, QT], BF16, tag="an", bufs=3,
                         name=f"an{pair}{qc}")
            nc.vector.tensor_mul(an[:], ous[1][0:64, :], rbs[1][:])
            nc.gpsimd.dma_start(
                a_t[64:128, pair, qc * QT:(qc + 1) * QT], an[:])

        # ---------------- filler unit machinery (no clock model) ----------
        filler = deque()     # items: (label, fn)
        emitted = set()
        state = {"done": 0}
        TOTAL_UNITS = (NKT * 4) + (NJT * 2 * 20) + (3 * NMT * 2) + 24  # 296

        def run_one():
            lab, fn = filler.popleft()
            fn()
            state["done"] += 1
            if lab is not None:
                emitted.add(lab)

        def drain_until(lab):
            while lab not in emitted:
                assert filler, f"filler ran dry before {lab}"
                run_one()

        def pace(frac):
            tgt = frac * TOTAL_UNITS
            while filler and state["done"] < tgt:
                run_one()

        # --- v projection units: 16 token-tiles, 4 chunks of 2 dc each ---
        def v_units(tlo, thi):
            for tt in range(tlo, thi):
                cell = {}

                def mk(tt, dlo, dhi, last):
                    def fn():
                        if dlo == 0:
                            cell["ps"] = ps_qkv.tile(
                                [128, QT], F32, tag="qv", name=f"psv{tt}")
                        ps = cell["ps"]
                        for dc in range(dlo, dhi):
                            nc.tensor.matmul(
                                ps[:],
                                xT_sb[:, tt // 4, dc,
                                      (tt % 4) * 128:(tt % 4 + 1) * 128],
                                wv_sb[:, dc, :],
                                start=(dc == 0), stop=(dc == NDC - 1))
                        if last:
                            nc.vector.tensor_copy(
                                v_aug[:, tt, :, 0:64],
                                ps[:].rearrange("p (h d) -> p h d", h=HG))
                    return fn
                for ci in range(4):
                    dlo, dhi = 2 * ci, 2 * ci + 2
                    lab = ("v", tt) if ci == 3 else None
                    yield (lab, mk(tt, dlo, dhi, ci == 3))

        # --- q/k projection + rope units for one (pair, half) ---
        def proj_half_units(pair, hf, swp_eng=None):
            preq = qw.tile([128, 1024], BF16, tag="preq",
                           name=f"preq{pair}{hf}")
            prek = qw.tile([128, 1024], BF16, tag="prek",
                           name=f"prek{pair}{hf}")
            swps = {}

            def mkp(w_sb, pre, tn, dlo, dhi, last, cell):
                def fn():
                    if dlo == 0:
                        cell["ps"] = ps_qkv.tile(
                            [128, QT], F32, tag="qv",
                            name=f"psp{pair}{tn}")
                    ps = cell["ps"]
                    proj_mms(pair, w_sb, ps, tn, dlo, dhi)
                    if last:
                        nc.vector.tensor_copy(
                            pre[:, (tn - 2 * hf) * QT:(tn - 2 * hf + 1) * QT],
                            ps[:])
                return fn

            def mkd(name, pre):
                def fn():
                    swps[name] = rope_dma(pair, name, pre, hf,
                                          dma_eng=swp_eng)
                return fn

            def mkm(name, pre, dst):
                def fn():
                    rope_mul(pair, pre, swps[name], dst, hf)
                return fn

            # tn-interleaved (q-tn0, k-tn0, q-tn1, k-tn1) to match the DMA
            # arrival order in the head phase (x-t0 + wq, then wk, then x-t1)
            for tn_i in range(2):
                for name, w_sb, pre in (("q", wq_sb, preq),
                                        ("k", wk_sb, prek)):
                    tn = 2 * hf + tn_i
                    cell = {}
                    for ci in range(4):
                        yield (None, mkp(w_sb, pre, tn, 2 * ci,
                                         2 * ci + 2, ci == 3, cell))
            for name, pre in (("q", preq), ("k", prek)):
                yield (None, mkd(name, pre))
            for name, pre, dst in (("q", preq, qrot), ("k", prek, krot)):
                yield (("rope" + name, pair, hf), mkm(name, pre, dst))

        # --- output projection units for one qc: 8 mt, 2 chunks of 2 jc ---
        def outproj_units(qc):
            for mt in range(NMT):
                cell = {}

                def mk(qc, mt, jlo, jhi, last):
                    def fn():
                        if jlo == 0:
                            cell["ps"] = ps_qkv.tile(
                                [128, QT], F32, tag="qv", name=f"op{qc}{mt}")
                        op = cell["ps"]
                        for jc in range(jlo, jhi):
                            nc.tensor.matmul(
                                op[:],
                                wo_sb[:, jc, mt * 128:(mt + 1) * 128],
                                a_t[:, jc, qc * QT:(qc + 1) * QT],
                                start=(jc == 0), stop=(jc == NJT - 1))
                        if last:
                            ot = sp.tile([128, QT], BF16, tag="ot", bufs=3,
                                         name=f"ot{qc}{mt}")
                            nc.vector.tensor_copy(ot[:], op[:])
                            nc.sync.dma_start(
                                outT[:, mt, qc * QT:(qc + 1) * QT],
                                ot[:])
                    return fn
                for ci in range(2):
                    yield (None, mk(qc, mt, 2 * ci, 2 * ci + 2, ci == 1))

        # final round's outproj is split: jc 0-2 accumulate to SBUF while the
        # last block still runs; only the 8 jc=3 matmuls wait on the final
        # evac chain, shrinking the kernel tail from ~8us to ~2.5us.
        ota_hold = {}

        def outproj_a_units(qc):
            for mt in range(NMT):
                def mk(qc, mt):
                    def fn():
                        op = ps_qkv.tile([128, QT], F32, tag="qv",
                                         name=f"opA{qc}{mt}")
                        for jc in range(3):
                            nc.tensor.matmul(
                                op[:],
                                wo_sb[:, jc, mt * 128:(mt + 1) * 128],
                                a_t[:, jc, qc * QT:(qc + 1) * QT],
                                start=(jc == 0), stop=(jc == 2))
                        ota = sp.tile([128, QT], F32, tag="ota", bufs=8,
                                      name=f"ota{mt}")
                        nc.vector.tensor_copy(ota[:], op[:])
                        ota_hold[mt] = ota
                    return fn
                yield (None, mk(qc, mt))

        def outproj_b_units(qc):
            for mt in range(NMT):
                def mk(qc, mt):
                    def fn():
                        op = ps_qkv.tile([128, QT], F32, tag="qv",
                                         name=f"opB{qc}{mt}")
                        nc.tensor.matmul(
                            op[:],
                            wo_sb[:, 3, mt * 128:(mt + 1) * 128],
                            a_t[:, 3, qc * QT:(qc + 1) * QT],
                            start=True, stop=True)
                        ot = sp.tile([128, QT], BF16, tag="ot", bufs=3,
                                     name=f"otB{qc}{mt}")
                        nc.vector.tensor_add(ot[:], op[:], ota_hold[mt][:])
                        nc.sync.dma_start(
                            outT[:, mt, qc * QT:(qc + 1) * QT], ot[:])
                    return fn
                yield (None, mk(qc, mt))

        # ------ phase 0: pair-0 first-half q/k projections + rope up front
        p0units = list(proj_half_units(0, 0, swp_eng=nc.gpsimd))

        def run_sync(units):
            for lab, fn in units:
                fn()
                state["done"] += 1
                if lab is not None:
                    emitted.add(lab)
        # emit q-tn0 + k-tn0 first (x-t0 + pair-0 weights land first); the
        # v-projection tiles tt0-3 then cover the wait for x-t1 before the
        # q/k-tn1 pieces
        run_sync(p0units[:8])

        # filler deque ordered so qc-major forced drains pull just-in-time:
        # round qc0 needs v tt0-3 + all pairs' half-0; round qc1 needs v
        # tt4-7; round qc2 needs half-1 rope + v tt8-11; round qc3 the rest.
        for u in v_units(0, 4):
            filler.append(u)
        for _ in range(16):      # v tt0-3 into the prologue
            run_one()
        run_sync(p0units[8:])
        for pair in (1, 2, 3):
            for u in proj_half_units(pair, 0):
                filler.append(u)
        for u in v_units(4, 8):
            filler.append(u)
        for u in proj_half_units(0, 1):
            filler.append(u)
        for u in v_units(8, 12):
            filler.append(u)
        for u in proj_half_units(1, 1):
            filler.append(u)
        for u in proj_half_units(2, 1):
            filler.append(u)
        for u in proj_half_units(3, 1):
            filler.append(u)
        for u in v_units(12, 16):
            filler.append(u)
        # outproj units are appended as each qc round's last evac_b fires

        # -------- main attention loop (qc-major, cross-block pipeline) ----
        # The AV+evac tail of each block is interleaved with the next
        # block's scores via a global pend queue, so the PE never drains
        # while the last exps of a block finish.
        hold = {"b": None, "age": 0}  # (pair, qc, ous, rbs) + chunks waited

        def fire_evac_b():
            bpair, bqc, ous, rbs = hold["b"]
            emit_evac_b(bpair, bqc, ous, rbs)
            hold["b"] = None
            if bqc == NQT - 1 and bpair == NJT - 2:
                for u in outproj_a_units(bqc):
                    filler.append(u)
            if bpair == NJT - 1:
                if bqc == NQT - 1:
                    gen = list(outproj_b_units(bqc))
                elif bqc == NQT - 2:
                    # hold back the last 6 mt so the PE stays warm (and
                    # busy) during the final block's evac chain
                    units = list(outproj_units(bqc))
                    tail_stash.extend(units[-16:])
                    gen = units[:-16]
                else:
                    gen = list(outproj_units(bqc))
                for u in gen:
                    filler.append(u)

        blocks = [(qc, pair) for qc in range(NQT) for pair in range(NJT)]
        pend = deque()   # (pair, qc, kc, e, ov0, ov1, last)
        tail_stash = []

        def flush_one():
            fpair, fqc, kc, e, ov0, ov1, last = pend.popleft()
            if fpair == 0:
                drain_until(("v", kc))
            emit_av(fpair, fqc, kc, e, ov0, ov1, last)
            if last:
                if hold["b"] is not None:   # short blocks: fire before reuse
                    fire_evac_b()
                hold["b"] = (fpair, fqc) + emit_evac_a(fpair, fqc, ov0, ov1)
                hold["age"] = 0

        gchunk = 0
        for bi, (qc, pair) in enumerate(blocks):
            ngrp = 4 * qc + 4      # k chunks covering causal range
            ov0 = ps_ov.tile([96, QT], F32, tag="ov", name=f"ov0_{pair}{qc}")
            ov1 = ps_ov.tile([96, QT], F32, tag="ov", name=f"ov1_{pair}{qc}")
            drain_until(("ropeq", pair, qc // 2))
            drain_until(("ropek", pair, 0))
            for kc in range(ngrp):
                if kc >= 8:
                    drain_until(("ropek", pair, 1))
                if pair == 0:
                    drain_until(("v", min(kc + 2, ngrp - 1)))
                st = emit_scores(pair, qc, kc)
                e = emit_exp(pair, qc, kc, st)
                pend.append((pair, qc, kc, e, ov0, ov1, kc == ngrp - 1))
                # fire the deferred normalize only once its broadcast DMA
                # chain (~4 chunks) has surely landed, so the DVE FIFO never
                # head-of-line blocks the masks queued behind it
                hold["age"] += 1
                if hold["b"] is not None and hold["age"] >= 6:
                    fire_evac_b()
                gchunk += 1
                pace((gchunk - 0.5) / NCHUNK)
                if len(pend) > 3:
                    flush_one()
                pace(gchunk / NCHUNK)
                # the final block is exp-bound once regular filler runs dry;
                # feed half the reserved stash through it, keep the rest for
                # the evac-chain tail
                if bi == len(blocks) - 1 and kc % 2 == 0 and                         len(tail_stash) > 8 and not filler:
                    filler.append(tail_stash.pop(0))
                    run_one()
                if kc in (1, 2) and bi + kc < len(blocks):
                    # pre-pull the rope (and projections) of the next TWO
                    # blocks so a block start never waits on the rope
                    # DMA+DVE chain, even across the short qc=0 blocks
                    nqc, npair = blocks[bi + kc]
                    drain_until(("ropeq", npair, nqc // 2))
                    drain_until(("ropek", npair, 0))
                    if nqc >= 2:
                        drain_until(("ropek", npair, 1))

        # drain the pipeline: remaining AVs with reserved filler interleaved;
        # the bulk of the reserve runs after the final evac_a so the PE stays
        # warm (and useful) under the final DVE evac chain
        while pend:
            if tail_stash:
                filler.append(tail_stash.pop(0))
            if filler:
                run_one()
            flush_one()
        for u in tail_stash:
            filler.append(u)
        tail_stash = []
        fire_evac_b()

        # drain whatever filler remains (tail output projections)
        while filler:
            run_one()

    nc.compile()
    return nc


_NC_CACHE = {}


def _get_nc():
    if "nc" not in _NC_CACHE:
        _NC_CACHE["nc"] = _build_nc()
    return _NC_CACHE["nc"]


def _host_prep(x, wq, wk, wv, wo, token_positions):
    head_perm = np.concatenate([np.arange(0, DK, 2), np.arange(1, DK, 2)])
    pos = np.asarray(token_positions).astype(np.float32)
    half = np.arange(0, DK, 2, dtype=np.float32) / DK
    inv_freq = THETA ** (-half)
    ang = pos[:, None] * inv_freq[None, :]        # [S, 32]
    cosT = np.cos(ang).T.astype(np.float32)       # [32, S]
    sinT = np.sin(ang).T.astype(np.float32)
    c128 = np.tile(cosT, (4, 1)).astype(bf16)     # [128, S]
    s128 = np.concatenate([-sinT, sinT, -sinT, sinT], 0).astype(bf16)

    kp = np.arange(128)[:, None, None]
    jj = np.arange(4)[None, :, None]
    qf = np.arange(QT)[None, None, :]
    maskd = (qf >= kp + 128 * jj).astype(bf16)    # [128, 4, QT]

    def chunk3(arr, nchunk):
        # [nchunk*128, F] -> [128, nchunk, F] (SBUF-native layout)
        f = arr.shape[1]
        return np.ascontiguousarray(
            arr.reshape(nchunk, 128, f).transpose(1, 0, 2))

    def prep_qk(w, g):
        rows = w.reshape(H, DK, D)[g * HG:(g + 1) * HG][:, head_perm]
        return np.ascontiguousarray(rows.reshape(HG * DK, D).T).astype(bf16)

    def prep_v(w, g):
        rows = w.reshape(H, DK, D)[g * HG:(g + 1) * HG]
        return np.ascontiguousarray(rows.reshape(HG * DK, D).T).astype(bf16)

    common = {"c128": c128, "s128": s128, "maskd": maskd}
    in_maps = []
    for c in range(NCORES):
        b, g = c // NG, c % NG
        m = dict(common)
        # [NQT, 128, NDC, QT]: token-major quarters, contiguous per partition
        m["xT"] = np.ascontiguousarray(
            x[b].T.astype(bf16).reshape(NDC, 128, NQT, QT)
            .transpose(2, 1, 0, 3))
        def pairmajor(arr):
            # [D, 512] -> [NJT, 128, NDC, 128]
            return np.ascontiguousarray(
                arr.reshape(NDC, 128, NJT, 128).transpose(2, 1, 0, 3))
        m["wqT"] = pairmajor(prep_qk(wq, g))
        m["wkT"] = pairmajor(prep_qk(wk, g))
        m["wvT"] = chunk3(prep_v(wv, g), NDC)
        m["woT"] = chunk3(
            np.ascontiguousarray(wo[:, g * HG * DK:(g + 1) * HG * DK].T
                                 ).astype(bf16), NJT)
        in_maps.append(m)
    return in_maps


def kernel(x, wq, wk, wv, wo, token_positions, _trace=False):
    x = np.asarray(x, dtype=np.float32)
    in_maps = _host_prep(x, wq, wk, wv, wo, token_positions)
    nc = _get_nc()
    res = run_bass_kernel_spmd(nc, in_maps, core_ids=list(range(NCORES)),
                               trace=_trace)
    out = np.zeros((B, S, D), np.float32)
    for b in range(B):
        # outT: [128, NMT, S] bf16 partials; row d of out.T is [mt*128+p]
        acc = (res.results[2 * b]["outT"].astype(np.float32) +
               res.results[2 * b + 1]["outT"].astype(np.float32))
        full = acc.transpose(1, 0, 2).reshape(D, S)
        out[b] = full.T
    if _trace:
        kernel.last_results = res
    return out
